# revision 10
# baseline (speedup 1.0000x reference)
"""Trainium2 Bass kernel for nn_AttentionBlock (GroupNorm + 1x1-conv QKV +
full self-attention over N=HW=4096 + output projection + residual).

Distribution: data-parallel over batch B=8, one batch element per NeuronCore.

v3: fp8 attention probabilities + DoubleRow matmuls.
  - S = K^T Q stays bf16 (PSUM-write bound at 1 col/cycle regardless).
  - exp(S - 1.5) is computed split across two engines per j-tile:
    ACT does columns [0:EA) via the Exp table with fp8e4 output (the -1.5
    shift avoids the TRN e4m3 +/-240->Inf ceiling and cancels in softmax);
    DVE does columns [EA:1024) via a Schraudolph bit-trick: one
    tensor_scalar computes rne(x*(8/ln2) + b) into a uint8 view of the
    fp8 tile (negative results saturate to 0 == fp8 +0).
  - O accumulates with perf_mode=DoubleRow: lhsT = V^T key-pairs
    [128,2,128] fp8, rhs = P pairs [128,2,512] fp8 -> 0.5 cyc/col (2x).
  - The softmax denominator comes from a second DoubleRow matmul with an
    all-ones stationary: out [128,512] = column sums of P replicated
    across all partitions (free broadcast), accumulated in PSUM over all
    key pairs. This removes the entire DVE accumulation tree.
  - Residual is folded on the PE: pp = I^T xb + wp^T onrm (xb = x+beff in
    bf16, identity stationary), so the output path is two pure
    PSUM->SBUF copies (split ACT/DVE) + DMA stores.
  - K bias is dropped entirely (a per-query constant in S, cancels in
    softmax); all of K/Q/V emission happens in the preamble.

PSUM: s0,s1 (S double-buffer) + o (O accum) + d (denominator accum) =
8 banks. The proj matmuls reuse the d banks in the inter-block window
between the reciprocal read and the next block's deferred d-matmuls.
"""

import numpy as np

B, C, H, W = 8, 128, 64, 64
HW = H * W                      # 4096
GROUPS = 8
GSIZE = C // GROUPS             # 16
EPS = 1e-5
NJ = HW // 128                  # 32 j-tiles
NJP = NJ // 2                   # 16 key pairs
QW = 1024                       # queries per block
NQT = HW // QW                  # 4 blocks
SCALE = float(C) ** -0.5
CSH = 1.5                       # exp shift, cancels in softmax
A8 = 11.541560327111707         # 8/ln2
B8 = 55.55 - CSH * A8           # schraudolph bias, rne-calibrated

_CACHE = {}


def _build():
    from contextlib import ExitStack

    import concourse.bacc as bacc
    import concourse.tile as tile
    from concourse import mybir

    f32 = mybir.dt.float32
    bf16 = mybir.dt.bfloat16
    fp8 = mybir.dt.float8e4
    u8 = mybir.dt.uint8
    AF = mybir.ActivationFunctionType
    PM = mybir.MatmulPerfMode
    ALU = mybir.AluOpType

    nc = bacc.Bacc("TRN2", target_bir_lowering=False, debug=False)

    x_in = nc.dram_tensor("x", [C, HW], f32, kind="ExternalInput")
    gamma_in = nc.dram_tensor("gamma", [C, 1], f32, kind="ExternalInput")
    beta_in = nc.dram_tensor("beta", [C, 1], f32, kind="ExternalInput")
    bq_in = nc.dram_tensor("bq", [C, 1], f32, kind="ExternalInput")
    beff_in = nc.dram_tensor("beff", [C, 1], f32, kind="ExternalInput")
    wq_in = nc.dram_tensor("wqT", [C, C], f32, kind="ExternalInput")
    wk_in = nc.dram_tensor("wkT", [C, C], f32, kind="ExternalInput")
    wv_in = nc.dram_tensor("wvT", [C, C], f32, kind="ExternalInput")
    wp_in = nc.dram_tensor("wpT", [C, C], f32, kind="ExternalInput")
    id_in = nc.dram_tensor("ident", [C, C], f32, kind="ExternalInput")
    ig_in = nc.dram_tensor("ig", [C, GROUPS], f32, kind="ExternalInput")
    igt_in = nc.dram_tensor("igt", [GROUPS, C], f32, kind="ExternalInput")
    out_dram = nc.dram_tensor("out", [C, HW], f32, kind="ExternalOutput")

    with tile.TileContext(nc) as tc, ExitStack() as ctx:
        const = ctx.enter_context(tc.tile_pool(name="const", bufs=1))
        big = ctx.enter_context(tc.tile_pool(name="big", bufs=1))
        stats = ctx.enter_context(tc.tile_pool(name="stats", bufs=1))
        ptpool = ctx.enter_context(tc.tile_pool(name="pt", bufs=12))
        osbp = ctx.enter_context(tc.tile_pool(name="osb", bufs=2))
        rcpp = ctx.enter_context(tc.tile_pool(name="rcp", bufs=2))
        onrmp = ctx.enter_context(tc.tile_pool(name="onrm", bufs=3))
        ostp = ctx.enter_context(tc.tile_pool(name="ostg", bufs=4))
        ps = ctx.enter_context(tc.tile_pool(name="ps", bufs=1, space="PSUM"))

        # ---------------- x load (4 DMA queues), consts ----------------
        x_sb = big.tile([C, HW], f32, tag="x")
        xq = [nc.sync, nc.scalar, nc.gpsimd]
        for ch in range(8):
            sl = slice(ch * 512, (ch + 1) * 512)
            xq[ch % 3].dma_start(x_sb[:, sl], x_in[:, sl])

        def cload(t_in, shape, tag):
            t = const.tile(shape, f32, tag=tag)
            nc.sync.dma_start(t[:], t_in[:])
            return t

        gamma = cload(gamma_in, [C, 1], "c_gamma")
        beta = cload(beta_in, [C, 1], "c_beta")
        bq = cload(bq_in, [C, 1], "c_bq")
        beff = cload(beff_in, [C, 1], "c_beff")
        ig = cload(ig_in, [C, GROUPS], "c_ig")
        igt = cload(igt_in, [GROUPS, C], "c_igt")
        wq_f = cload(wq_in, [C, C], "c_wq_f")
        wk_f = cload(wk_in, [C, C], "c_wk_f")
        wv_f = cload(wv_in, [C, C], "c_wv_f")
        wp_f = cload(wp_in, [C, C], "c_wp_f")
        id_f = cload(id_in, [C, C], "c_id_f")

        with nc.allow_low_precision(reason="bf16 weights: rel tol is 2e-2"):
            wq_b = const.tile([C, C], bf16)
            nc.vector.tensor_copy(wq_b[:], wq_f[:])
            wk_b = const.tile([C, C], bf16)
            nc.vector.tensor_copy(wk_b[:], wk_f[:])
            wv_b = const.tile([C, C], bf16)
            nc.vector.tensor_copy(wv_b[:], wv_f[:])
            wp_b = const.tile([C, C], bf16)
            nc.vector.tensor_copy(wp_b[:], wp_f[:])
            id_b = const.tile([C, C], bf16)
            nc.vector.tensor_copy(id_b[:], id_f[:])
            ones8 = const.tile([128, 2, C], fp8)
            nc.gpsimd.memset(ones8[:], 1.0)

        negc = const.tile([C, 1], f32)
        nc.gpsimd.memset(negc[:], -CSH)
        eps_t = const.tile([GROUPS, 1], f32)
        nc.vector.memset(eps_t[:], EPS)
        magic_t = const.tile([GROUPS, 1], mybir.dt.uint32)
        nc.vector.memset(magic_t[:], 0x5F3759DF)
        c15_t = const.tile([GROUPS, 1], f32)
        nc.vector.memset(c15_t[:], 1.5)

        # ---------------- groupnorm stats via bn_stats ----------------
        bnst = stats.tile([C, 8, 6], f32)
        for ch in range(8):
            sl = slice(ch * 512, (ch + 1) * 512)
            nc.vector.bn_stats(bnst[:, ch, :], x_sb[:, sl])
        mv = stats.tile([C, 2], f32)
        nc.vector.bn_aggr(mv[:], bnst[:])
        warm = stats.tile([GROUPS, 1], f32)
        nc.scalar.activation(warm[:], eps_t[:], AF.Exp)

        msq = stats.tile([C, 2], f32)
        nc.vector.tensor_copy(msq[:, 0:1], mv[:, 0:1])
        nc.vector.tensor_mul(msq[:, 1:2], mv[:, 0:1], mv[:, 0:1])
        nc.vector.tensor_add(msq[:, 1:2], msq[:, 1:2], mv[:, 1:2])

        # persistent PSUM tiles: the whole loop's working set (8 banks)
        s_ps = [None, None]

        def fetch_s(k):
            s_ps[k] = ps.tile([C, QW], f32, tag=f"s{k}", name=f"s{k}")
            return s_ps[k]

        fetch_s(0)
        fetch_s(1)

        gs_ps = s_ps[0][0:GROUPS, 0:2]
        nc.tensor.matmul(gs_ps, ig[:], msq[:], start=True, stop=True)
        gmr = stats.tile([GROUPS, 2], f32)
        nc.vector.tensor_copy(gmr[:, 0:1], gs_ps[:, 0:1])
        gmsq = stats.tile([GROUPS, 1], f32)
        nc.vector.tensor_mul(gmsq[:], gmr[:, 0:1], gmr[:, 0:1])
        gve = stats.tile([GROUPS, 1], f32)
        nc.vector.tensor_sub(gve[:], gs_ps[:, 1:2], gmsq[:])
        nc.vector.tensor_scalar(
            gve[:], gve[:], eps_t[:], None, ALU.add
        )
        # rstd = rsqrt(var+eps): quake guess + 1 Newton step
        u32 = mybir.dt.uint32
        gu = stats.tile([GROUPS, 1], u32)
        nc.vector.tensor_scalar(
            gu[:], gve[:].bitcast(u32), 1, None,
            ALU.logical_shift_right,
        )
        nc.vector.tensor_sub(gu[:], magic_t[:], gu[:])
        gy = stats.tile([GROUPS, 1], f32)
        nc.vector.tensor_copy(gy[:], gu[:].bitcast(f32))
        gh = stats.tile([GROUPS, 1], f32)
        nc.vector.tensor_scalar_mul(gh[:], gve[:], 0.5)
        gt = stats.tile([GROUPS, 1], f32)
        nc.vector.tensor_mul(gt[:], gy[:], gy[:])
        nc.vector.tensor_mul(gt[:], gt[:], gh[:])
        nc.vector.tensor_sub(gt[:], c15_t[:], gt[:])
        nc.vector.tensor_mul(gmr[:, 1:2], gy[:], gt[:])

        bc_ps = s_ps[1][0:C, 0:2]
        nc.tensor.matmul(bc_ps, igt[:], gmr[:], start=True, stop=True)
        a_c = stats.tile([C, 1], f32)
        b_c = stats.tile([C, 1], f32)
        tmc = stats.tile([C, 1], f32)
        nc.vector.tensor_scalar_mul(a_c[:], gamma[:], bc_ps[:, 1:2])
        nc.vector.tensor_scalar_mul(tmc[:], a_c[:], bc_ps[:, 0:1])
        nc.vector.tensor_sub(b_c[:], beta[:], tmc[:])

        # ---------------- hn, xb, and all of K/Q/V ----------------
        hn = big.tile([C, HW], bf16, tag="hn")
        q_r = big.tile([C, HW], bf16, tag="q")
        k_r = big.tile([C, HW], bf16, tag="k")
        v_nat = big.tile([C, HW], bf16, tag="vnat")
        vt_b = big.tile([128, NJ, 128], bf16, tag="vtb")
        vtp = big.tile([128, NJP, 2, 128], fp8, tag="vtp")
        xb = big.tile([C, HW], bf16, tag="xb")

        lp = nc.allow_low_precision(reason="fp8/bf16 data path: tol 2e-2")
        lp.__enter__()

        # hn halves alternate DVE/Pool; xb (= x + beff, bf16) on Pool
        for h in range(8):
            sl = slice(h * 512, (h + 1) * 512)
            eng = nc.vector if h % 2 == 0 else nc.gpsimd
            eng.tensor_scalar(
                hn[:, sl], x_sb[:, sl], a_c[:], b_c[:], ALU.mult, ALU.add
            )
        for ch in range(4):
            sl = slice(ch * 1024, (ch + 1) * 1024)
            nc.gpsimd.tensor_scalar(
                xb[:, sl], x_sb[:, sl], beff[:], None, ALU.add
            )

        # 24 psum stagings rotate through 8 half-bank slots
        o_ps_pre = ps.tile([C, QW], f32, tag="o")
        d_ps_pre = ps.tile([C, QW], f32, tag="d")
        slots = []
        for t in (s_ps[0], s_ps[1], o_ps_pre, d_ps_pre):
            slots.append(t[:, 0:512])
            slots.append(t[:, 512:1024])
        sidx = 0

        def stage():
            nonlocal sidx
            s = slots[sidx % 8]
            sidx += 1
            return s

        # K: plain copy (k-bias is a per-query constant in S -> cancels)
        for h in range(8):
            sl = slice(h * 512, (h + 1) * 512)
            stg = stage()
            nc.tensor.matmul(stg, wk_b[:], hn[:, sl], start=True, stop=True)
            nc.scalar.activation(k_r[:, sl], stg, AF.Copy)
        # Q: bias bq (pre-scaled) via ACT/DVE alternating
        for h in range(8):
            sl = slice(h * 512, (h + 1) * 512)
            stg = stage()
            nc.tensor.matmul(stg, wq_b[:], hn[:, sl], start=True, stop=True)
            if h % 2 == 0:
                nc.scalar.activation(
                    q_r[:, sl], stg, AF.Identity, bias=bq[:]
                )
            else:
                nc.vector.tensor_scalar(
                    q_r[:, sl], stg, bq[:], None, ALU.add
                )
        # V natural layout; xbar DMA transposes (bf16) into vt_b
        vq = [nc.sync, nc.scalar]
        for h in range(8):
            sl = slice(h * 512, (h + 1) * 512)
            stg = stage()
            nc.tensor.matmul(stg, wv_b[:], hn[:, sl], start=True, stop=True)
            if h % 2 == 0:
                nc.scalar.activation(v_nat[:, sl], stg, AF.Copy)
            else:
                nc.vector.tensor_copy(v_nat[:, sl], stg)
            for t in range(4):
                nt = h * 4 + t
                vq[nt % 2].dma_start_transpose(
                    vt_b[:, nt, :], v_nat[:, nt * 128:(nt + 1) * 128]
                )

        # ---------------- main attention loop ----------------
        # ACT exp column share per j; reduced on js where the block chain
        # puts extra work on ACT/DVE.
        chain = {}
        pend_d = []     # deferred denominator DR-matmul groups
        d_cnt = [0]     # groups issued for current block's d accumulation
        cur_d = [None]  # current block's d accumulator (lazy fetch: the
                        # banks are vacated by the prev block's proj first)
        cur_qt = [0]

        def flush_d(maxn):
            n = 0
            while pend_d and n < maxn:
                if cur_d[0] is None:
                    cur_d[0] = d_ps_pre if cur_qt[0] == 0 else \
                        ps.tile([C, QW], f32, tag="d", name="d_acc")
                pt, first = pend_d.pop(0)
                for cch in range(2):
                    csl = slice(cch * 512, (cch + 1) * 512)
                    nc.tensor.matmul(
                        cur_d[0][:, csl], ones8[:], pt[:, :, csl],
                        start=first, stop=(d_cnt[0] == NJP - 1),
                        perf_mode=PM.DoubleRow,
                    )
                d_cnt[0] += 1
                n += 1

        def emit_O(o_ps, jp, pt):
            for cch in range(2):
                csl = slice(cch * 512, (cch + 1) * 512)
                nc.tensor.matmul(
                    o_ps[:, csl], vtp[:, jp, :, :], pt[:, :, csl],
                    start=(jp == 0), stop=(jp == NJP - 1),
                    perf_mode=PM.DoubleRow,
                )

        def chain_ops(qt, j):
            # normalization + projection + store for block qt-1,
            # emitted at fixed js of block qt (d banks reused for proj)
            pq = qt - 1
            if j == 2:
                rc = rcpp.tile([128, QW], f32)
                nc.vector.reciprocal(rc[:, 0:512], chain["d_prev"][:, 0:512])
                chain["rc"] = rc
            elif j == 4:
                nc.vector.reciprocal(
                    chain["rc"][:, 512:1024], chain["d_prev"][:, 512:1024]
                )
            elif j in (5, 6):
                cch = j - 5
                csl = slice(cch * 512, (cch + 1) * 512)
                onrm = onrmp.tile([C, 512], bf16)
                nc.gpsimd.tensor_tensor(
                    onrm[:], chain["osb_prev"][:, csl], chain["rc"][:, csl],
                    ALU.mult,
                )
                chain[f"onrm{cch}"] = onrm
            elif j in (7, 9):
                cch = (j - 7) // 2
                csl = slice(cch * 512, (cch + 1) * 512)
                xsl = slice(pq * QW + cch * 512, pq * QW + (cch + 1) * 512)
                pp = chain["d_prev"][:, csl]
                nc.tensor.matmul(pp, id_b[:], xb[:, xsl], start=True,
                                 stop=False)
                nc.tensor.matmul(pp, wp_b[:], chain[f"onrm{cch}"][:],
                                 start=False, stop=True)
            elif j in (8, 10):
                cch = (j - 8) // 2
                csl = slice(cch * 512, (cch + 1) * 512)
                osl = slice(pq * QW + cch * 512, pq * QW + (cch + 1) * 512)
                ost = ostp.tile([C, 512], f32)
                if cch == 0:
                    nc.scalar.activation(ost[:], chain["d_prev"][:, csl],
                                         AF.Copy)
                else:
                    nc.vector.tensor_copy(ost[:], chain["d_prev"][:, csl])
                nc.sync.dma_start(out_dram[:, osl], ost[:])

        def ea_of(qt, j):
            if qt > 0 and j in (2, 4):
                return 768      # DVE busy with reciprocal
            if qt > 0 and j in (8, 10):
                return 256      # DVE/ACT busy with ost copies
            return 512

        for qt in range(NQT):
            o_ps = ps.tile([C, QW], f32, tag="o", name="o_acc") if qt > 0 else o_ps_pre
            o_sb = osbp.tile([C, QW], bf16)
            pts = {}
            d_cnt[0] = 0
            cur_d[0] = None
            cur_qt[0] = qt
            for j in range(NJ):
                jp = j // 2
                qoff = qt * QW

                # --- S pair
                sp = ps.tile([C, QW], f32, tag=f"s{j % 2}", name=f"sp{j % 2}")
                nc.tensor.matmul(
                    sp[:, 0:512], k_r[:, j * 128:(j + 1) * 128],
                    q_r[:, qoff:qoff + 512], start=True, stop=True,
                )
                nc.tensor.matmul(
                    sp[:, 512:1024], k_r[:, j * 128:(j + 1) * 128],
                    q_r[:, qoff + 512:qoff + 1024], start=True, stop=True,
                )
                # --- pipelined O(jp-1) between the two S pairs
                if j % 2 == 0 and jp >= 1:
                    emit_O(o_ps, jp - 1, pts[jp - 1])
                # --- deferred denominator matmuls (after proj vacates d)
                if j >= 16:
                    flush_d(2)

                # --- exp split ACT/DVE
                if j % 2 == 0:
                    pt = ptpool.tile([128, 2, QW], fp8)
                    pts[jp] = pt
                else:
                    pt = pts[jp]
                i = j % 2
                ea = ea_of(qt, j)
                nc.scalar.activation(
                    pt[:, i, 0:ea], sp[:, 0:ea], AF.Exp, bias=negc[:]
                )
                nc.vector.tensor_scalar(
                    pt[:, i, ea:QW].bitcast(u8), sp[:, ea:QW],
                    A8, B8, ALU.mult, ALU.add,
                )
                if j % 2 == 1:
                    pend_d.append((pt, jp == 0))

                # --- aux: vtp fp8 copies (block 0), prev-block chain
                if qt == 0 and j < 16:
                    for t in range(2):
                        nt = 2 * j + t
                        nc.gpsimd.tensor_copy(
                            vtp[:, nt // 2, nt % 2, :], vt_b[:, nt, :]
                        )
                if qt > 0:
                    chain_ops(qt, j)

            # ---- block epilogue ----
            emit_O(o_ps, NJP - 1, pts[NJP - 1])
            flush_d(99)
            for cch in range(2):
                csl = slice(cch * 512, (cch + 1) * 512)
                nc.scalar.activation(o_sb[:, csl], o_ps[:, csl], AF.Copy)
            chain["d_prev"] = cur_d[0]
            chain["osb_prev"] = o_sb
            if qt == NQT - 1:
                # final tail: run the qt=3 chain inline
                for j in (2, 4, 5, 6, 7, 8, 9, 10):
                    chain_ops(qt + 1, j)

        lp.__exit__(None, None, None)

    nc.compile()
    return nc


def _get_nc():
    if "nc" not in _CACHE:
        _CACHE["nc"] = _build()
    return _CACHE["nc"]


def _prep_inputs(x, gamma, beta, w_qkv, b_qkv, w_proj, b_proj):
    x = np.ascontiguousarray(x, dtype=np.float32)
    w_qkv = np.asarray(w_qkv, dtype=np.float32)
    b_qkv = np.asarray(b_qkv, dtype=np.float32)
    w_proj = np.asarray(w_proj, dtype=np.float32)
    b_proj = np.asarray(b_proj, dtype=np.float32)

    wq = w_qkv[0:C, :]
    wk = w_qkv[C:2 * C, :]
    wv = w_qkv[2 * C:3 * C, :]
    bqv = b_qkv[0:C]
    bvv = b_qkv[2 * C:3 * C]

    wqT = np.ascontiguousarray((wq * SCALE).T)
    wkT = np.ascontiguousarray(wk.T)
    wvT = np.ascontiguousarray(wv.T)
    wpT = np.ascontiguousarray(w_proj.T)
    beff = (b_proj + w_proj @ bvv).astype(np.float32)

    ig = np.zeros((C, GROUPS), np.float32)
    ig[np.arange(C), np.arange(C) // GSIZE] = 1.0
    igt = np.ascontiguousarray(ig.T)
    ig = ig * (1.0 / GSIZE)

    common = {
        "gamma": np.asarray(gamma, np.float32).reshape(C, 1),
        "beta": np.asarray(beta, np.float32).reshape(C, 1),
        "bq": (bqv * SCALE).reshape(C, 1),
        "beff": beff.reshape(C, 1),
        "wqT": wqT,
        "wkT": wkT,
        "wvT": wvT,
        "wpT": wpT,
        "ident": np.eye(C, dtype=np.float32),
        "ig": ig,
        "igt": igt,
    }
    in_maps = []
    for b in range(B):
        m = dict(common)
        m["x"] = np.ascontiguousarray(x[b].reshape(C, HW))
        in_maps.append(m)
    return in_maps


def kernel(x, gamma, beta, w_qkv, b_qkv, w_proj, b_proj):
    from concourse.bass_utils import run_bass_kernel_spmd

    nc = _get_nc()
    in_maps = _prep_inputs(x, gamma, beta, w_qkv, b_qkv, w_proj, b_proj)
    res = run_bass_kernel_spmd(nc, in_maps, list(range(B)))
    out = np.stack([res.results[b]["out"] for b in range(B)], axis=0)
    return out.reshape(B, C, H, W).astype(np.float32)


# revision 14
# speedup vs baseline: 1.4092x; 1.4092x over previous
"""Trainium2 Bass kernel for nn_AttentionBlock (GroupNorm + 1x1-conv QKV +
full self-attention over N=HW=4096 + output projection + residual).

Distribution: data-parallel over batch B=8, one batch element per NeuronCore.

v3: fp8 attention probabilities + DoubleRow matmuls.
  - S = K^T Q stays bf16 (PSUM-write bound at 1 col/cycle regardless).
  - exp(S - 1.5) is computed split across two engines per j-tile:
    ACT does columns [0:EA) via the Exp table with fp8e4 output (the -1.5
    shift avoids the TRN e4m3 +/-240->Inf ceiling and cancels in softmax);
    DVE does columns [EA:1024) via a Schraudolph bit-trick: one
    tensor_scalar computes rne(x*(8/ln2) + b) into a uint8 view of the
    fp8 tile (negative results saturate to 0 == fp8 +0).
  - O accumulates with perf_mode=DoubleRow: lhsT = V^T key-pairs
    [128,2,128] fp8, rhs = P pairs [128,2,512] fp8 -> 0.5 cyc/col (2x).
  - The softmax denominator comes from a second DoubleRow matmul with an
    all-ones stationary: out [128,512] = column sums of P replicated
    across all partitions (free broadcast), accumulated in PSUM over all
    key pairs. This removes the entire DVE accumulation tree.
  - Residual is folded on the PE: pp = I^T xb + wp^T onrm (xb = x+beff in
    bf16, identity stationary), so the output path is two pure
    PSUM->SBUF copies (split ACT/DVE) + DMA stores.
  - K bias is dropped entirely (a per-query constant in S, cancels in
    softmax); all of K/Q/V emission happens in the preamble.

PSUM: s0,s1 (S double-buffer) + o (O accum) + d (denominator accum) =
8 banks. The proj matmuls reuse the d banks in the inter-block window
between the reciprocal read and the next block's deferred d-matmuls.
"""

import numpy as np

B, C, H, W = 8, 128, 64, 64
HW = H * W                      # 4096
GROUPS = 8
GSIZE = C // GROUPS             # 16
EPS = 1e-5
NJ = HW // 128                  # 32 j-tiles
NJP = NJ // 2                   # 16 key pairs
QW = 1024                       # queries per block
NQT = HW // QW                  # 4 blocks
SCALE = float(C) ** -0.5
CSH = 1.5                       # exp shift, cancels in softmax
A8 = 11.541560327111707         # 8/ln2
B8 = 55.55 - CSH * A8           # schraudolph bias, rne-calibrated

_CACHE = {}


def _build():
    from contextlib import ExitStack

    import concourse.bacc as bacc
    import concourse.tile as tile
    from concourse import mybir

    f32 = mybir.dt.float32
    bf16 = mybir.dt.bfloat16
    fp8 = mybir.dt.float8e4
    u8 = mybir.dt.uint8
    AF = mybir.ActivationFunctionType
    PM = mybir.MatmulPerfMode
    ALU = mybir.AluOpType

    nc = bacc.Bacc("TRN2", target_bir_lowering=False, debug=False)

    x_in = nc.dram_tensor("x", [C, HW], f32, kind="ExternalInput")
    gamma_in = nc.dram_tensor("gamma", [C, 1], f32, kind="ExternalInput")
    beta_in = nc.dram_tensor("beta", [C, 1], f32, kind="ExternalInput")
    bq_in = nc.dram_tensor("bq", [C, 1], f32, kind="ExternalInput")
    beff_in = nc.dram_tensor("beff", [C, 1], f32, kind="ExternalInput")
    wq_in = nc.dram_tensor("wqT", [C, C], f32, kind="ExternalInput")
    wk_in = nc.dram_tensor("wkT", [C, C], f32, kind="ExternalInput")
    wv_in = nc.dram_tensor("wvT", [C, C], f32, kind="ExternalInput")
    wp_in = nc.dram_tensor("wpT", [C, C], f32, kind="ExternalInput")
    id_in = nc.dram_tensor("ident", [C, C], f32, kind="ExternalInput")
    ig_in = nc.dram_tensor("ig", [C, GROUPS], f32, kind="ExternalInput")
    igt_in = nc.dram_tensor("igt", [GROUPS, C], f32, kind="ExternalInput")
    out_dram = nc.dram_tensor("out", [C, HW], f32, kind="ExternalOutput")

    with tile.TileContext(nc) as tc, ExitStack() as ctx:
        const = ctx.enter_context(tc.tile_pool(name="const", bufs=1))
        big = ctx.enter_context(tc.tile_pool(name="big", bufs=1))
        stats = ctx.enter_context(tc.tile_pool(name="stats", bufs=1))
        ptpool = ctx.enter_context(tc.tile_pool(name="pt", bufs=12))
        osbp = ctx.enter_context(tc.tile_pool(name="osb", bufs=2))
        rcpp = ctx.enter_context(tc.tile_pool(name="rcp", bufs=2))
        onrmp = ctx.enter_context(tc.tile_pool(name="onrm", bufs=3))
        ostp = ctx.enter_context(tc.tile_pool(name="ostg", bufs=4))
        ps = ctx.enter_context(tc.tile_pool(name="ps", bufs=1, space="PSUM"))

        # ---------------- x load (4 DMA queues), consts ----------------
        x_sb = big.tile([C, HW], f32, tag="x")
        xq = [nc.sync, nc.scalar, nc.gpsimd]
        for ch in range(8):
            sl = slice(ch * 512, (ch + 1) * 512)
            xq[ch % 3].dma_start(x_sb[:, sl], x_in[:, sl])

        def cload(t_in, shape, tag):
            t = const.tile(shape, f32, tag=tag)
            nc.gpsimd.dma_start(t[:], t_in[:])
            return t

        gamma = cload(gamma_in, [C, 1], "c_gamma")
        beta = cload(beta_in, [C, 1], "c_beta")
        bq = cload(bq_in, [C, 1], "c_bq")
        beff = cload(beff_in, [C, 1], "c_beff")
        ig = cload(ig_in, [C, GROUPS], "c_ig")
        igt = cload(igt_in, [GROUPS, C], "c_igt")
        wq_f = cload(wq_in, [C, C], "c_wq_f")
        wk_f = cload(wk_in, [C, C], "c_wk_f")
        wv_f = cload(wv_in, [C, C], "c_wv_f")
        wp_f = cload(wp_in, [C, C], "c_wp_f")
        id_f = cload(id_in, [C, C], "c_id_f")

        with nc.allow_low_precision(reason="fp8 ones/bias consts"):
            ones8 = const.tile([128, 2, C], fp8)
            nc.gpsimd.memset(ones8[:], 1.0)
            ones_r = const.tile([1, C], bf16)
            nc.gpsimd.memset(ones_r[:], 1.0)

        negc = const.tile([C, 1], f32)
        nc.gpsimd.memset(negc[:], -CSH)
        eps_t = const.tile([GROUPS, 1], f32)
        nc.vector.memset(eps_t[:], EPS)
        magic_t = const.tile([GROUPS, 1], mybir.dt.uint32)
        nc.vector.memset(magic_t[:], 0x5F3759DF)
        c15_t = const.tile([GROUPS, 1], f32)
        nc.vector.memset(c15_t[:], 1.5)

        # ---------------- groupnorm stats via bn_stats ----------------
        bnst = stats.tile([C, 8, 6], f32)
        for ch in range(8):
            sl = slice(ch * 512, (ch + 1) * 512)
            nc.vector.bn_stats(bnst[:, ch, :], x_sb[:, sl])
        mv = stats.tile([C, 2], f32)
        nc.vector.bn_aggr(mv[:], bnst[:])
        warm = stats.tile([GROUPS, 1], f32)
        nc.scalar.activation(warm[:], eps_t[:], AF.Exp)

        msq = stats.tile([C, 2], f32)
        nc.vector.tensor_copy(msq[:, 0:1], mv[:, 0:1])
        nc.vector.tensor_mul(msq[:, 1:2], mv[:, 0:1], mv[:, 0:1])
        nc.vector.tensor_add(msq[:, 1:2], msq[:, 1:2], mv[:, 1:2])

        # persistent PSUM tiles: the whole loop's working set (8 banks)
        s_ps = [None, None]

        def fetch_s(k):
            s_ps[k] = ps.tile([C, QW], f32, tag=f"s{k}", name=f"s{k}")
            return s_ps[k]

        fetch_s(0)
        fetch_s(1)

        gs_ps = s_ps[0][0:GROUPS, 0:2]
        nc.tensor.matmul(gs_ps, ig[:], msq[:], start=True, stop=True)
        gmr = stats.tile([GROUPS, 2], f32)
        nc.vector.tensor_copy(gmr[:, 0:1], gs_ps[:, 0:1])
        gmsq = stats.tile([GROUPS, 1], f32)
        nc.vector.tensor_mul(gmsq[:], gmr[:, 0:1], gmr[:, 0:1])
        gve = stats.tile([GROUPS, 1], f32)
        nc.vector.tensor_sub(gve[:], gs_ps[:, 1:2], gmsq[:])
        nc.vector.tensor_scalar(
            gve[:], gve[:], eps_t[:], None, ALU.add
        )
        # rstd = rsqrt(var+eps): quake guess + 1 Newton step
        u32 = mybir.dt.uint32
        gu = stats.tile([GROUPS, 1], u32)
        nc.vector.tensor_scalar(
            gu[:], gve[:].bitcast(u32), 1, None,
            ALU.logical_shift_right,
        )
        nc.vector.tensor_sub(gu[:], magic_t[:], gu[:])
        gy = stats.tile([GROUPS, 1], f32)
        nc.vector.tensor_copy(gy[:], gu[:].bitcast(f32))
        gh = stats.tile([GROUPS, 1], f32)
        nc.vector.tensor_scalar_mul(gh[:], gve[:], 0.5)
        gt = stats.tile([GROUPS, 1], f32)
        nc.vector.tensor_mul(gt[:], gy[:], gy[:])
        nc.vector.tensor_mul(gt[:], gt[:], gh[:])
        nc.vector.tensor_sub(gt[:], c15_t[:], gt[:])
        nc.vector.tensor_mul(gmr[:, 1:2], gy[:], gt[:])

        bc_ps = s_ps[1][0:C, 0:2]
        nc.tensor.matmul(bc_ps, igt[:], gmr[:], start=True, stop=True)
        a_c = stats.tile([C, 1], f32)
        b_c = stats.tile([C, 1], f32)
        tmc = stats.tile([C, 1], f32)
        nc.vector.tensor_scalar_mul(a_c[:], gamma[:], bc_ps[:, 1:2])
        nc.vector.tensor_scalar_mul(tmc[:], a_c[:], bc_ps[:, 0:1])
        nc.vector.tensor_sub(b_c[:], beta[:], tmc[:])

        with nc.allow_low_precision(reason="bf16 weights: rel tol is 2e-2"):
            wk_b = const.tile([C, C], bf16)
            nc.vector.tensor_copy(wk_b[:], wk_f[:])
            wq_b = const.tile([C, C], bf16)
            nc.vector.tensor_copy(wq_b[:], wq_f[:])
            wv_b = const.tile([C, C], bf16)
            nc.vector.tensor_copy(wv_b[:], wv_f[:])
            wp_b = const.tile([C, C], bf16)
            nc.vector.tensor_copy(wp_b[:], wp_f[:])
            id_b = const.tile([C, C], bf16)
            nc.vector.tensor_copy(id_b[:], id_f[:])

        # ---------------- hn, and all of K/Q/V (preamble) ----------------
        hn = big.tile([C, HW], bf16, tag="hn")
        q_r = big.tile([C, HW], bf16, tag="q")
        k_r = big.tile([C, HW], bf16, tag="k")
        vtp = big.tile([128, HW], fp8, tag="vtp")  # V^T, key-major
        xb = big.tile([C, HW], bf16, tag="xb")

        lp = nc.allow_low_precision(reason="fp8/bf16 data path: tol 2e-2")
        lp.__enter__()

        # hn halves: ACT does odd (affine via scale+bias), DVE even
        for h in range(8):
            sl = slice(h * 512, (h + 1) * 512)
            if h % 2 == 1:
                nc.scalar.activation(
                    hn[:, sl], x_sb[:, sl], AF.Identity,
                    bias=b_c[:], scale=a_c[:],
                )
            else:
                nc.vector.tensor_scalar(
                    hn[:, sl], x_sb[:, sl], a_c[:], b_c[:],
                    ALU.mult, ALU.add,
                )

        # 24 psum stagings rotate through 8 half-bank slots
        o_ps_pre = ps.tile([C, QW], f32, tag="o")
        d_ps_pre = ps.tile([C, QW], f32, tag="d")
        slots = []
        for t in (s_ps[0], s_ps[1], o_ps_pre, d_ps_pre):
            slots.append(t[:, 0:512])
            slots.append(t[:, 512:1024])
        sidx = 0

        def stage():
            nonlocal sidx
            st = slots[sidx % 8]
            sidx += 1
            return st

        # K: plain copy (k-bias is a per-query constant in S -> cancels)
        # Q: bias bq (pre-scaled);  V^T: direct per-tile matmuls, fp8 copy
        for h in range(8):
            sl = slice(h * 512, (h + 1) * 512)
            stg = stage()
            nc.tensor.matmul(stg, wk_b[:], hn[:, sl], start=True, stop=True)
            nc.scalar.activation(k_r[:, sl], stg, AF.Copy)
            stg = stage()
            nc.tensor.matmul(stg, wq_b[:], hn[:, sl], start=True, stop=True)
            if h % 2 == 0:
                nc.scalar.activation(
                    q_r[:, sl], stg, AF.Identity, bias=bq[:]
                )
            else:
                nc.vector.tensor_scalar(
                    q_r[:, sl], stg, bq[:], None, ALU.add
                )
            stg = stage()
            for t in range(4):
                nt = h * 4 + t
                nc.tensor.matmul(
                    stg[:, t * 128:(t + 1) * 128],
                    hn[:, nt * 128:(nt + 1) * 128], wv_b[:],
                    start=True, stop=True,
                )
            if h % 2 == 1:
                nc.scalar.activation(vtp[:, sl], stg, AF.Copy)
            else:
                nc.vector.tensor_copy(vtp[:, sl], stg)

        def vtp_pair(jp):
            return vtp[:, jp * 256:(jp + 1) * 256].rearrange(
                "p (two f) -> p two f", two=2)

        # xb = x + beff (bf16), for the PE-side residual; not needed until
        # the first block chain, so it goes after K/Q/V
        for ch in range(4):
            sl = slice(ch * 1024, (ch + 1) * 1024)
            if ch % 2 == 0:
                nc.scalar.activation(
                    xb[:, sl], x_sb[:, sl], AF.Identity, bias=beff[:]
                )
            else:
                nc.vector.tensor_scalar(
                    xb[:, sl], x_sb[:, sl], beff[:], None, ALU.add
                )

        # ---------------- main attention loop ----------------
        # ACT exp column share per j; reduced on js where the block chain
        # puts extra work on ACT/DVE.
        chain = {}
        pend_d = []     # deferred denominator DR-matmul groups
        d_cnt = [0]     # groups issued for current block's d accumulation
        cur_d = [None]  # current block's d accumulator (lazy fetch: the
                        # banks are vacated by the prev block's proj first)
        cur_qt = [0]

        def flush_d(maxn):
            n = 0
            while pend_d and n < maxn:
                if cur_d[0] is None:
                    cur_d[0] = d_ps_pre if cur_qt[0] == 0 else \
                        ps.tile([C, QW], f32, tag="d", name="d_acc")
                pt, first = pend_d.pop(0)
                for cch in range(2):
                    csl = slice(cch * 512, (cch + 1) * 512)
                    nc.tensor.matmul(
                        cur_d[0][:, csl], ones8[:], pt[:, :, csl],
                        start=first, stop=(d_cnt[0] == NJP - 1),
                        perf_mode=PM.DoubleRow,
                    )
                d_cnt[0] += 1
                n += 1

        def emit_O(o_ps, jp, pt):
            for cch in range(2):
                csl = slice(cch * 512, (cch + 1) * 512)
                nc.tensor.matmul(
                    o_ps[:, csl], vtp_pair(jp), pt[:, :, csl],
                    start=(jp == 0), stop=(jp == NJP - 1),
                    perf_mode=PM.DoubleRow,
                )

        def chain_ops(qt, j):
            # normalization + projection + store for block qt-1, emitted at
            # fixed js of block qt.  Reciprocal runs on a DMA-scattered
            # [128,8] layout (DVE reciprocal is ~6.6 cyc/elem); the result
            # row is broadcast back across partitions by a PE matmul into
            # the vacated d banks, which the proj then reuses.
            pq = qt - 1
            dpv = chain["d_prev"]
            if j in (2, 3):
                cch = j - 2
                csl = slice(cch * 512, (cch + 1) * 512)
                if cch == 0:
                    drow = rcpp.tile([1, QW], f32)
                    chain["drow"] = drow
                    rs = rcpp.tile([128, 8], f32, tag="rs", name="rs")
                    chain["rs"] = rs
                    nc.scalar.activation(drow[0:1, csl], dpv[0:1, csl],
                                         AF.Copy)
                else:
                    drow = chain["drow"]
                    nc.vector.tensor_copy(drow[0:1, csl], dpv[0:1, csl])
                nc.sync.dma_start(
                    chain["rs"][:, cch * 4:(cch + 1) * 4], drow[0:1, csl])
            elif j in (4, 5):
                cch = j - 4
                if cch == 0:
                    rc = rcpp.tile([128, 8], bf16, tag="rc", name="rc")
                    chain["rc"] = rc
                    rrow = rcpp.tile([1, QW], bf16, tag="rrow", name="rrow")
                    chain["rrow"] = rrow
                nc.vector.reciprocal(
                    chain["rc"][:, cch * 4:(cch + 1) * 4],
                    chain["rs"][:, cch * 4:(cch + 1) * 4])
                nc.sync.dma_start(
                    chain["rrow"][0:1, cch * 512:(cch + 1) * 512],
                    chain["rc"][:, cch * 4:(cch + 1) * 4])
            elif j in (6, 7):
                cch = j - 6
                csl = slice(cch * 512, (cch + 1) * 512)
                nc.tensor.matmul(
                    dpv[:, csl], ones_r[:], chain["rrow"][0:1, csl],
                    start=True, stop=True,
                )
            elif j in (8, 9):
                cch = j - 8
                csl = slice(cch * 512, (cch + 1) * 512)
                onrm = onrmp.tile([C, 512], bf16)
                nc.vector.tensor_tensor(
                    onrm[:], chain["osb_prev"][:, csl], dpv[:, csl],
                    ALU.mult,
                )
                chain[f"onrm{cch}"] = onrm
            elif j in (10, 11):
                cch = j - 10
                csl = slice(cch * 512, (cch + 1) * 512)
                xsl = slice(pq * QW + cch * 512, pq * QW + (cch + 1) * 512)
                pp = dpv[:, csl]
                nc.tensor.matmul(pp, id_b[:], xb[:, xsl], start=True,
                                 stop=False)
                nc.tensor.matmul(pp, wp_b[:], chain[f"onrm{cch}"][:],
                                 start=False, stop=True)
            elif j in (12, 13):
                cch = j - 12
                csl = slice(cch * 512, (cch + 1) * 512)
                osl = slice(pq * QW + cch * 512, pq * QW + (cch + 1) * 512)
                ost = ostp.tile([C, 512], f32)
                if cch == 0:
                    nc.scalar.activation(ost[:], dpv[:, csl], AF.Copy)
                else:
                    nc.vector.tensor_copy(ost[:], dpv[:, csl])
                nc.sync.dma_start(out_dram[:, osl], ost[:])

        def ea_of(qt, j):
            if qt > 0:
                if j == 2:
                    return 320      # ACT does the drow copy
                if j == 3:
                    return 768      # DVE does drow + reciprocal next
                if j in (8, 9):
                    return 768      # DVE does onrm
                if j == 12:
                    return 256      # ACT ost copy
                if j == 13:
                    return 768      # DVE ost copy
            return 512

        for qt in range(NQT):
            o_ps = ps.tile([C, QW], f32, tag="o", name="o_acc") if qt > 0 else o_ps_pre
            o_sb = osbp.tile([C, QW], bf16)
            pts = {}
            d_cnt[0] = 0
            cur_d[0] = None
            cur_qt[0] = qt
            for j in range(NJ):
                jp = j // 2
                qoff = qt * QW

                # --- S pair
                sp = ps.tile([C, QW], f32, tag=f"s{j % 2}", name=f"sp{j % 2}")
                nc.tensor.matmul(
                    sp[:, 0:512], k_r[:, j * 128:(j + 1) * 128],
                    q_r[:, qoff:qoff + 512], start=True, stop=True,
                )
                nc.tensor.matmul(
                    sp[:, 512:1024], k_r[:, j * 128:(j + 1) * 128],
                    q_r[:, qoff + 512:qoff + 1024], start=True, stop=True,
                )
                # --- pipelined O(jp-1) between the two S pairs
                if j % 2 == 0 and jp >= 1:
                    emit_O(o_ps, jp - 1, pts[jp - 1])
                # --- deferred denominator matmuls (after proj vacates d)
                if j >= 16:
                    flush_d(2)

                # --- exp split ACT/DVE
                if j % 2 == 0:
                    pt = ptpool.tile([128, 2, QW], fp8)
                    pts[jp] = pt
                else:
                    pt = pts[jp]
                i = j % 2
                ea = ea_of(qt, j)
                nc.scalar.activation(
                    pt[:, i, 0:ea], sp[:, 0:ea], AF.Exp, bias=negc[:]
                )
                nc.vector.tensor_scalar(
                    pt[:, i, ea:QW].bitcast(u8), sp[:, ea:QW],
                    A8, B8, ALU.mult, ALU.add,
                )
                if j % 2 == 1:
                    pend_d.append((pt, jp == 0))

                # --- aux: vtp fp8 copies (block 0), prev-block chain
                if qt > 0:
                    chain_ops(qt, j)

            # ---- block epilogue ----
            emit_O(o_ps, NJP - 1, pts[NJP - 1])
            flush_d(99)
            for cch in range(2):
                csl = slice(cch * 512, (cch + 1) * 512)
                nc.scalar.activation(o_sb[:, csl], o_ps[:, csl], AF.Copy)
            chain["d_prev"] = cur_d[0]
            chain["osb_prev"] = o_sb
            if qt == NQT - 1:
                # final tail: run the qt=3 chain inline
                for j in (2, 3, 4, 5, 6, 7, 8, 9, 10, 11, 12, 13):
                    chain_ops(qt + 1, j)

        lp.__exit__(None, None, None)

    nc.compile()
    return nc


def _get_nc():
    if "nc" not in _CACHE:
        _CACHE["nc"] = _build()
    return _CACHE["nc"]


def _prep_inputs(x, gamma, beta, w_qkv, b_qkv, w_proj, b_proj):
    x = np.ascontiguousarray(x, dtype=np.float32)
    w_qkv = np.asarray(w_qkv, dtype=np.float32)
    b_qkv = np.asarray(b_qkv, dtype=np.float32)
    w_proj = np.asarray(w_proj, dtype=np.float32)
    b_proj = np.asarray(b_proj, dtype=np.float32)

    wq = w_qkv[0:C, :]
    wk = w_qkv[C:2 * C, :]
    wv = w_qkv[2 * C:3 * C, :]
    bqv = b_qkv[0:C]
    bvv = b_qkv[2 * C:3 * C]

    wqT = np.ascontiguousarray((wq * SCALE).T)
    wkT = np.ascontiguousarray(wk.T)
    wvT = np.ascontiguousarray(wv.T)
    wpT = np.ascontiguousarray(w_proj.T)
    beff = (b_proj + w_proj @ bvv).astype(np.float32)

    ig = np.zeros((C, GROUPS), np.float32)
    ig[np.arange(C), np.arange(C) // GSIZE] = 1.0
    igt = np.ascontiguousarray(ig.T)
    ig = ig * (1.0 / GSIZE)

    common = {
        "gamma": np.asarray(gamma, np.float32).reshape(C, 1),
        "beta": np.asarray(beta, np.float32).reshape(C, 1),
        "bq": (bqv * SCALE).reshape(C, 1),
        "beff": beff.reshape(C, 1),
        "wqT": wqT,
        "wkT": wkT,
        "wvT": wvT,
        "wpT": wpT,
        "ident": np.eye(C, dtype=np.float32),
        "ig": ig,
        "igt": igt,
    }
    in_maps = []
    for b in range(B):
        m = dict(common)
        m["x"] = np.ascontiguousarray(x[b].reshape(C, HW))
        in_maps.append(m)
    return in_maps


def kernel(x, gamma, beta, w_qkv, b_qkv, w_proj, b_proj):
    from concourse.bass_utils import run_bass_kernel_spmd

    nc = _get_nc()
    in_maps = _prep_inputs(x, gamma, beta, w_qkv, b_qkv, w_proj, b_proj)
    res = run_bass_kernel_spmd(nc, in_maps, list(range(B)))
    out = np.stack([res.results[b]["out"] for b in range(B)], axis=0)
    return out.reshape(B, C, H, W).astype(np.float32)
